# revision 1
# baseline (speedup 1.0000x reference)
"""Trainium2 Bass kernel for DiceFromLabelsLoss (histogram binning).

Strategy: data-parallel over the flattened voxel dim across 8 cores (each
core gets half of one sample). Per core, 27 class-masks (c_pred 1..9,
intersection via w = 11*yp + yt == 12c, c_true 1..9) are built on the DVE
as plain bf16 is_equal passes (4x mode), and reduced by the TensorEngine:
a ones[128,1] stationary matmul column-sums each mask slab into a
per-class PSUM slice, PSUM-accumulating across slabs and chunks. PSUM
slices live at partitions {0,32,64,96} (tile_position col-groups) x 7
bank slots. A final DVE reduce drains PSUM to a [128, 8] tile, DMA'd out;
the host does the tiny final dice reduction.

accum_out (TensorScalarPtrReduce) is deliberately NOT used: measured on
HW it is ~10x slower than a plain tensor_scalar pass.
"""

import numpy as np

NUM_CLASSES = 10
N_CORES = 8
SHAPE = (4, 1, 160, 160, 160)
N_SAMPLES = 4
V_TOTAL = 4 * 160 * 160 * 160          # 16_384_000
V_CORE = V_TOTAL // N_CORES            # 2_048_000
P = 128
F = V_CORE // P                        # 16000
NCHUNK = 2
FC = F // NCHUNK                       # 8000
MM_N = 500                             # matmul slab width (psum slot pitch 512)
CHUNK_PLAN = [(0, 2000), (2000, 6000), (8000, 8000)]  # (offset, size) in F cols
N_CLS = 27
N_FOLD = 9                             # masks pre-folded on DVE before the PE

ACT_CLS = {19, 20, 22, 23, 26}

_CACHE = {}


def _build_bass(repeat=1, variant="full"):
    import concourse.bacc as bacc
    import concourse.mybir as mybir
    import concourse.tile as tile

    nc = bacc.Bacc(None, target_bir_lowering=False)
    yp_d = nc.dram_tensor("yp", [P, F], mybir.dt.int32, kind="ExternalInput")
    yt_d = nc.dram_tensor("yt", [P, F], mybir.dt.int32, kind="ExternalInput")
    out_d = nc.dram_tensor("out", [P, 8], mybir.dt.float32, kind="ExternalOutput")

    eq = mybir.AluOpType.is_equal
    bf16 = mybir.dt.bfloat16
    f32 = mybir.dt.float32
    n_slabs = FC // MM_N

    with tile.TileContext(nc) as tc:
        with (
            tc.tile_pool(name="io", bufs=2) as io_pool,
            tc.tile_pool(name="work", bufs=2) as work_pool,
            tc.tile_pool(name="mask", bufs=3) as mask_pool,
            tc.tile_pool(name="fold", bufs=2) as fold_pool,
            tc.tile_pool(name="act", bufs=2) as act_pool,
            tc.tile_pool(name="act1", bufs=1) as act1_pool,
            tc.tile_pool(name="acc", bufs=1) as acc_pool,
            tc.tile_pool(name="psum", bufs=1, space="PSUM") as psum_pool,
        ):
            ones = acc_pool.tile([P, 1], bf16)
            nc.gpsimd.memset(ones[:], 1.0)
            sqb = acc_pool.tile([P, N_CLS], f32)
            for i2 in range(N_CLS):
                if i2 in ACT_CLS:
                    cv = (i2 - 18 + 1) if i2 >= 18 else 0
                    nc.gpsimd.memset(sqb[:, i2:i2 + 1], -float(cv))
            acc = acc_pool.tile([P, 8], f32)
            nc.gpsimd.memset(acc[:], 0.0)
            # one psum tile spanning 7 banks; class i uses
            # [32*(i%4) : 32*(i%4)+1, 512*(i//4) : 512*(i//4)+MM_N]
            psum = psum_pool.tile([P, 7 * 512], f32)

            chunk_plan = CHUNK_PLAN
            n_chunks = len(chunk_plan)
            for k0 in range(n_chunks * repeat):
                k = k0 % n_chunks
                off, FCk = chunk_plan[k]
                ypc = io_pool.tile([P, FC], bf16, tag="ypc")
                ytc = io_pool.tile([P, FC], bf16, tag="ytc")
                ypc = ypc[:, :FCk]
                ytc = ytc[:, :FCk]
                # SWDGE dma casts int32 -> bf16 during the transfer
                nc.gpsimd.dma_start(ypc[:], yp_d[:, off:off + FCk])
                nc.gpsimd.dma_start(ytc[:], yt_d[:, off:off + FCk])

                # w = 11*yp + yt in [0, 109]; intersection_c == count(w == 12c)
                w = work_pool.tile([P, FC], bf16, tag="w")
                w = w[:, :FCk]
                nc.vector.scalar_tensor_tensor(
                    out=w[:], in0=ypc[:], scalar=11.0, in1=ytc[:],
                    op0=mybir.AluOpType.mult, op1=mybir.AluOpType.add,
                )

                jobs = (
                    [(ypc, float(c)) for c in range(1, 10)]
                    + [(w, float(12 * c)) for c in range(1, 10)]
                    + [(ytc, float(c)) for c in range(1, 10)]
                )
                for i, (src, cval) in enumerate(jobs):
                    if variant == "pe_only":
                        if k0 == 0 and i == 0:
                            pe_mask = acc_pool.tile([P, FC], bf16)
                            _CACHE["pe_mask"] = pe_mask
                            nc.vector.tensor_scalar(
                                out=_CACHE["pe_mask"][:], in0=src[:],
                                scalar1=cval, scalar2=0.0,
                                op0=eq, op1=mybir.AluOpType.add,
                            )
                        mask = _CACHE["pe_mask"][:, :FCk]
                    elif i in ACT_CLS and variant == "full":
                        # ScalarE 2-pass mask: relu(1 - (v - c)^2)
                        at = act1_pool.tile([P, FC], bf16, tag="actt")
                        at = at[:, :FCk]
                        nc.scalar.activation(
                            out=at[:], in_=src[:],
                            func=mybir.ActivationFunctionType.Square,
                            bias=sqb[:, i:i + 1],
                        )
                        mask = act_pool.tile([P, FC], bf16, tag="actm")
                        mask = mask[:, :FCk]
                        nc.scalar.activation(
                            out=mask[:], in_=at[:],
                            func=mybir.ActivationFunctionType.Relu,
                            bias=1.0, scale=-1.0,
                        )
                    else:
                        mask = mask_pool.tile([P, FC], bf16, tag="mask")
                        mask = mask[:, :FCk]
                        nc.vector.tensor_scalar(
                            out=mask[:], in0=src[:], scalar1=cval, scalar2=0.0,
                            op0=eq, op1=mybir.AluOpType.add,
                        )
                    if variant == "dve_only":
                        if k0 == NCHUNK * repeat - 1 and i == N_CLS - 1:
                            nc.tensor.matmul(
                                psum[0:1, 0:MM_N], ones[:], mask[:, 0:MM_N],
                                start=True, stop=True, tile_position=(0, 0),
                            )
                        continue
                    if (i % 4 == 1 or i in (3, 7)) and i not in ACT_CLS:
                        # DVE folds mask halves ({0,1,2} values) to halve
                        # the PE stream for this class
                        fm = fold_pool.tile([P, FC // 2], bf16, tag="fmask")
                        fm = fm[:, :FCk // 2]
                        nc.vector.tensor_tensor(
                            out=fm[:], in0=mask[:, :FCk // 2],
                            in1=mask[:, FCk // 2:], op=mybir.AluOpType.add,
                        )
                        feed, fw = fm, FCk // 2
                    else:
                        feed, fw = mask, FCk
                    grp, slot = i % 4, i // 4
                    prow = 32 * grp
                    for s in range(fw // MM_N):
                        nc.tensor.matmul(
                            psum[prow:prow + 1,
                                 512 * slot:512 * slot + MM_N],
                            ones[:],
                            feed[:, s * MM_N:(s + 1) * MM_N],
                            start=(k == 0 and s == 0),
                            stop=(k == n_chunks - 1 and s == fw // MM_N - 1),
                            tile_position=(0, prow),
                        )

            # drain PSUM: per partition-group, reduce [1, 7, MM_N] -> [1, 7]
            for grp in range(4):
                prow = 32 * grp
                view = psum[prow:prow + 1, :].rearrange(
                    "p (b n) -> p b n", n=512
                )[:, :, 0:MM_N]
                nc.vector.tensor_reduce(
                    out=acc[prow:prow + 1, 0:7], in_=view,
                    axis=mybir.AxisListType.X, op=mybir.AluOpType.add,
                )
            nc.sync.dma_start(out_d[:], acc[:])
    nc.finalize()
    return nc


def _get_built():
    if "nc" not in _CACHE:
        _CACHE["nc"] = _build_bass()
    return _CACHE["nc"]


def _decode_counts(raw):
    """raw: [P, 8] per-core output -> (cp[9], ct[9], inter[9])."""
    vals = np.zeros(N_CLS, np.float64)
    for i in range(N_CLS):
        vals[i] = raw[32 * (i % 4), i // 4]
    return vals[0:9], vals[18:27], vals[9:18]


def _host_finish(per_core_raw):
    cp = np.zeros((N_SAMPLES, 9), np.float64)
    ct = np.zeros((N_SAMPLES, 9), np.float64)
    it = np.zeros((N_SAMPLES, 9), np.float64)
    cores_per_sample = N_CORES // N_SAMPLES
    for core, raw in enumerate(per_core_raw):
        s = core // cores_per_sample
        a, b, c = _decode_counts(raw)
        cp[s] += a
        ct[s] += b
        it[s] += c
    denom = cp + ct
    nonzero = denom > 0
    denom_safe = np.where(nonzero, denom, 1.0)
    dice_terms = np.where(nonzero, 2.0 * it / denom_safe, 0.0)
    weight = ct / ct.sum(-1, keepdims=True) / N_SAMPLES
    loss = 1.0 - np.sum(np.where(nonzero, weight, 0.0) * dice_terms)
    return np.array(loss, dtype=np.float32)


def _make_in_maps(y_pred, y_true):
    yp = np.ascontiguousarray(np.asarray(y_pred).reshape(-1)).astype(
        np.int32, copy=False
    )
    yt = np.ascontiguousarray(np.asarray(y_true).reshape(-1)).astype(
        np.int32, copy=False
    )
    in_maps = []
    for core in range(N_CORES):
        sl = slice(core * V_CORE, (core + 1) * V_CORE)
        in_maps.append({
            "yp": yp[sl].reshape(P, F),
            "yt": yt[sl].reshape(P, F),
        })
    return in_maps


def _run(in_maps, **kw):
    from concourse.bass_utils import run_bass_kernel_spmd

    nc = _get_built()
    res = run_bass_kernel_spmd(nc, in_maps, core_ids=list(range(N_CORES)), **kw)
    per_core = [r["out"] for r in res.results]
    return per_core, res


def kernel(y_pred, y_true):
    per_core, _ = _run(_make_in_maps(y_pred, y_true))
    return _host_finish(per_core)


if __name__ == "__main__":
    rng = np.random.default_rng(0)
    a = rng.integers(0, 10, SHAPE, dtype=np.int32)
    b = rng.integers(0, 10, SHAPE, dtype=np.int32)
    print(kernel(a, b))



# revision 5
# speedup vs baseline: 1.4751x; 1.4751x over previous
"""Trainium2 Bass kernel for DiceFromLabelsLoss (histogram binning), v2.

Strategy (per core, data-parallel over flattened voxels, 8 cores):

Host packs the two int32 label tensors losslessly into two int16 codes
  j1 = yp + 256*yt    (low byte yp, high byte yt)
  j2 = yt + 256*yp
so the device reads 4 bytes/voxel instead of 8, with no DMA cast
(HWDGE, GPSIMD left idle). Every histogram functional is then a single
4x-mode DVE tensor_scalar pass over one packed stream:
  intersection_c = count(j1 == 257c)
  S^t_c = count(yt >= c) = count(j1 >= 256c)   (ct via differencing)
  S^p_c = count(yp >= c) = count(j2 >= 256c)
The bf16 {0,1} masks are reduced by the PE via 4 concurrent
column-group matmuls (tile_position=(0,32a), ones stationary), each
class accumulating into its own 125-col PSUM slot (pitch 128) at
partition 32a -- one tensor_reduce drains all classes at the end.

ScalarE takes the high cumulative classes (c=7,8,9 of both histograms)
as steep-sigmoid soft steps, sigmoid(30(v-c+0.5)), reading the low
bytes of j1/j2 with stride 2; accum_out yields the per-partition sums
for free (verified exact: sigmoid saturates to exactly 0.0/1.0).

Host sums the tiny per-core outputs, differences the cumulatives, and
computes the weighted dice loss in float64.
"""

import numpy as np

NUM_CLASSES = 10
N_CORES = 8
SHAPE = (4, 1, 160, 160, 160)
N_SAMPLES = 4
V_TOTAL = 4 * 160 * 160 * 160          # 16_384_000
V_CORE = V_TOTAL // N_CORES            # 2_048_000
P = 128
F = V_CORE // P                        # 16000
NCHUNK = 2
FC = F // NCHUNK                       # 8000
SLAB = 125                             # psum cols used per class slot
PITCH = 128                            # psum col pitch per class slot
SIG_K = 30.0

# DVE/PE class slots
INTER_CLS = list(range(1, 10))         # j1 == 257c          slots 0..8
CT_DVE = list(range(1, 7))             # j1 >= 256c (yt>=c)  slots 9..14
CP_DVE = list(range(1, 7))             # j2 >= 256c (yp>=c)  slots 15..20
N_SLOTS = len(INTER_CLS) + len(CT_DVE) + len(CP_DVE)   # 21

# ACT classes (steep sigmoid on strided low bytes), accum slots
CT_ACT = [7, 8, 9]                     # on yt = low byte of j2
CP_ACT = [7, 8, 9]                     # on yp = low byte of j1
N_ACT = len(CT_ACT) + len(CP_ACT)      # 6

_CACHE = {}


def _build_bass(repeat=1):
    import concourse.bacc as bacc
    import concourse.mybir as mybir
    import concourse.tile as tile

    nc = bacc.Bacc(None, target_bir_lowering=False)
    j1_d = nc.dram_tensor("j1", [P, F], mybir.dt.int16, kind="ExternalInput")
    j2_d = nc.dram_tensor("j2", [P, F], mybir.dt.int16, kind="ExternalInput")
    cnt_d = nc.dram_tensor("cnt", [P, N_SLOTS], mybir.dt.float32,
                           kind="ExternalOutput")
    acc_d = nc.dram_tensor("acc", [P, N_ACT * NCHUNK], mybir.dt.float32,
                           kind="ExternalOutput")

    f32 = mybir.dt.float32
    bf16 = mybir.dt.bfloat16
    i16 = mybir.dt.int16
    op = mybir.AluOpType
    TS = nc.vector.tensor_scalar
    NR = NCHUNK * repeat

    # (slot, src_index, alu_op, scalar1)
    dve_specs = []
    for i, c in enumerate(INTER_CLS):
        dve_specs.append((i, 0, op.is_equal, 257.0 * c))
    for i, c in enumerate(CT_DVE):
        dve_specs.append((9 + i, 0, op.is_ge, 256.0 * c))
    for i, c in enumerate(CP_DVE):
        dve_specs.append((15 + i, 1, op.is_ge, 256.0 * c))

    # (acc_index, src_index, class) -- src: 0 -> yp (low byte of j1),
    # 1 -> yt (low byte of j2)
    act_specs = []
    for i, c in enumerate(CT_ACT):
        act_specs.append((i, 1, c))
    for i, c in enumerate(CP_ACT):
        act_specs.append((len(CT_ACT) + i, 0, c))

    with tile.TileContext(nc) as tc:
        with (
            tc.tile_pool(name="io", bufs=2) as io_pool,
            tc.tile_pool(name="mask", bufs=3) as mask_pool,
            tc.tile_pool(name="acc", bufs=1) as acc_pool,
            tc.tile_pool(name="psum", bufs=1, space="PSUM") as psum_pool,
        ):
            ones = acc_pool.tile([P, 1], bf16)
            nc.vector.memset(ones[:], 1.0)
            # per-ACT-class biases: -K*(c-0.5)
            biases = acc_pool.tile([P, N_ACT], f32)
            for ai, (_, _, c) in enumerate(act_specs):
                nc.vector.memset(biases[:, ai:ai + 1], -SIG_K * (c - 0.5))
            accs = acc_pool.tile([P, N_ACT * NCHUNK], f32)
            act_sink = acc_pool.tile([P, FC], bf16)
            zeros = acc_pool.tile([P, 512], bf16)
            nc.vector.memset(zeros[:], 0.0)

            psum = psum_pool.tile([P, N_SLOTS * PITCH], f32)
            # start=True clears has_written for the WHOLE bank, so chains
            # sharing a bank would wipe each other. Clear every used bank
            # once with a zero matmul; all class chains then accumulate
            # with start=False (per-element has_written semantics).
            n_banks = (N_SLOTS * PITCH + 511) // 512
            for a in range(4):
                for b in range(n_banks):
                    lo = 512 * b
                    hi = min(512 * (b + 1), N_SLOTS * PITCH)
                    nc.tensor.matmul(
                        psum[32 * a:32 * a + 1, lo:hi],
                        ones[:, 0:1], zeros[:, 0:hi - lo],
                        start=True, stop=False,
                        tile_position=(0, 32 * a),
                        skip_group_check=True,
                    )

            for k0 in range(NR):
                k = k0 % NCHUNK
                off = k * FC
                j1 = io_pool.tile([P, FC], i16, tag="j1")
                j2 = io_pool.tile([P, FC], i16, tag="j2")
                nc.sync.dma_start(j1[:], j1_d[:, off:off + FC])
                nc.sync.dma_start(j2[:], j2_d[:, off:off + FC])

                # ScalarE: steep-sigmoid cumulative counts, self-reduced
                for ai, (aidx, srci, c) in enumerate(act_specs):
                    src = [j1, j2][srci]
                    low = src[:].bitcast(mybir.dt.int8).rearrange(
                        "p (f two) -> p f two", two=2)[:, :, 0:1].squeeze(-1)
                    nc.scalar.activation(
                        out=act_sink[:], in_=low,
                        func=mybir.ActivationFunctionType.Sigmoid,
                        scale=SIG_K, bias=biases[:, aidx:aidx + 1],
                        accum_out=accs[:, aidx * NCHUNK + k:
                                       aidx * NCHUNK + k + 1],
                    )

                # DVE masks + PE column-group reduction
                q = FC // 4
                nslab = q // SLAB
                for slot, srci, o0, s1 in dve_specs:
                    src = [j1, j2][srci]
                    m = mask_pool.tile([P, FC], bf16, tag="mask")
                    TS(out=m[:], in0=src[:], scalar1=s1, scalar2=0.0,
                       op0=o0, op1=op.add)
                    # col-group inner so consecutive MMs land on different
                    # array column groups and overlap (tile concurrency)
                    for s in range(nslab):
                        for a in range(4):
                            lo = a * q
                            nc.tensor.matmul(
                                psum[32 * a:32 * a + 1,
                                     PITCH * slot:PITCH * slot + SLAB],
                                ones[:, 0:1],
                                m[:, lo + s * SLAB:lo + (s + 1) * SLAB],
                                start=False,
                                stop=(k0 == NR - 1 and s == nslab - 1),
                                tile_position=(0, 32 * a),
                                skip_group_check=True,
                            )

            cnt = acc_pool.tile([P, N_SLOTS], f32)
            view = psum[:, :].rearrange(
                "p (c n) -> p c n", n=PITCH)[:, :, 0:SLAB]
            nc.vector.tensor_reduce(
                out=cnt[:], in_=view,
                axis=mybir.AxisListType.X, op=op.add,
            )
            nc.sync.dma_start(cnt_d[:], cnt[:])
            nc.sync.dma_start(acc_d[:], accs[:])
    nc.finalize()
    return nc


def _get_built():
    if "nc" not in _CACHE:
        _CACHE["nc"] = _build_bass()
    return _CACHE["nc"]


def _make_in_maps(y_pred, y_true):
    yp = np.ascontiguousarray(np.asarray(y_pred).reshape(-1)).astype(
        np.int32, copy=False)
    yt = np.ascontiguousarray(np.asarray(y_true).reshape(-1)).astype(
        np.int32, copy=False)
    in_maps = []
    for core in range(N_CORES):
        sl = slice(core * V_CORE, (core + 1) * V_CORE)
        a = yp[sl].reshape(P, F)
        b = yt[sl].reshape(P, F)
        j1 = (a + (b << 8)).astype(np.int16)
        j2 = (b + (a << 8)).astype(np.int16)
        in_maps.append({"j1": j1, "j2": j2})
    return in_maps


def _run(in_maps, **kw):
    from concourse.bass_utils import run_bass_kernel_spmd

    nc = _get_built()
    res = run_bass_kernel_spmd(nc, in_maps, core_ids=list(range(N_CORES)),
                               **kw)
    per_core = [(r["cnt"], r["acc"]) for r in res.results]
    return per_core, res


def _decode_core(cnt, acc):
    """-> (inter[10], S_t[11], S_p[11]) cumulative counts for one core."""
    counts = np.zeros(N_SLOTS, np.float64)
    for slot in range(N_SLOTS):
        counts[slot] = sum(cnt[32 * a, slot] for a in range(4))
    inter = np.zeros(10, np.float64)
    S_t = np.zeros(11, np.float64)
    S_p = np.zeros(11, np.float64)
    S_t[0] = V_CORE
    S_p[0] = V_CORE
    for i, c in enumerate(INTER_CLS):
        inter[c] = counts[i]
    for i, c in enumerate(CT_DVE):
        S_t[c] = counts[9 + i]
    for i, c in enumerate(CP_DVE):
        S_p[c] = counts[15 + i]
    a = acc.astype(np.float64).reshape(P, N_ACT, NCHUNK).sum(axis=(0, 2))
    for i, c in enumerate(CT_ACT):
        S_t[c] = a[i]
    for i, c in enumerate(CP_ACT):
        S_p[c] = a[len(CT_ACT) + i]
    return inter, S_t, S_p


def _host_finish(per_core):
    cores_per_sample = N_CORES // N_SAMPLES
    it = np.zeros((N_SAMPLES, 10), np.float64)
    St = np.zeros((N_SAMPLES, 11), np.float64)
    Sp = np.zeros((N_SAMPLES, 11), np.float64)
    for core, (cnt, acc) in enumerate(per_core):
        s = core // cores_per_sample
        i, st, sp = _decode_core(cnt, acc)
        it[s] += i
        St[s] += st
        Sp[s] += sp
    # counts per class from cumulatives (S[10] == 0)
    ct = St[:, 1:10] - St[:, 2:11]     # classes 1..9
    cp = Sp[:, 1:10] - Sp[:, 2:11]
    inter = it[:, 1:10]

    denom = cp + ct
    nonzero = denom > 0
    denom_safe = np.where(nonzero, denom, 1.0)
    dice_terms = np.where(nonzero, 2.0 * inter / denom_safe, 0.0)
    weight = ct / ct.sum(-1, keepdims=True) / N_SAMPLES
    loss = 1.0 - np.sum(np.where(nonzero, weight, 0.0) * dice_terms)
    return np.array(loss, dtype=np.float32)


def kernel(y_pred, y_true):
    per_core, _ = _run(_make_in_maps(y_pred, y_true))
    return _host_finish(per_core)


if __name__ == "__main__":
    rng = np.random.default_rng(0)
    a = rng.integers(0, 10, SHAPE, dtype=np.int32)
    b = rng.integers(0, 10, SHAPE, dtype=np.int32)
    print(kernel(a, b))
